# revision 14
# baseline (speedup 1.0000x reference)
"""Trainium2 Bass kernel for nn_DLGeneEmbeddings.

Math (separable linear):
    y[b, j] = w_x * x[b, j] + (nongene[b] . W_ng + bias) + (emb[j] . W_e)
with
    nongene = x[:, G:G+64], W = [W_ng(64) | w_x(1) | W_e(32)].

Sharding: data-parallel over batch across 8 cores; each core gets 128 rows
of x (exactly the 128 SBUF partitions); emb / W / b replicated.

Per-core device kernel, work spread over four engines so the DMA stream
(~21 MB at ~358 GB/s) stays the bottleneck:
  GPSIMD: emb * W_e elementwise, indicator build, W|b broadcast load
  DVE:    reduces (ng term, gene term), final y = t + C add from PSUM
  ACT:    t = Identity(x * w_x + ngb)  (per-partition scale+bias)
  PE:     C[m, n] = sum_p ind[p, gg, m] * gtp[p, n] = gtp[gg, n]
          (K=80 indicator matmul broadcasting a gene-term row into PSUM)
  DMA:    x loads on the SP HWDGE ring, y stores on the ACT HWDGE ring.
"""

import numpy as np
from contextlib import ExitStack

import concourse.bass as bass
import concourse.bacc as bacc
import concourse.tile as tile
from concourse import mybir
from concourse.bass_utils import run_bass_kernel_spmd

F32 = mybir.dt.float32

B = 1024
G = 20000
DNG = 64
E = 32
IN_DIM = G + DNG          # 20064
FC_IN = DNG + 1 + E       # 97
NCORES = 8
PB = B // NCORES          # 128 rows per core == SBUF partitions

DMA_COLS = 2500           # 128 x 2500 x f32 = 1.25 MB per streaming DMA
NT = 500                  # compute tile (one PSUM bank)
EP = 80                   # partitions holding the emb table
EN = G // EP              # 250 genes per partition, contiguous
NQ = DMA_COLS // NT       # subtiles per DMA chunk


def build_kernel(nc: bass.Bass):
    xs = nc.dram_tensor("xs", [PB, IN_DIM], F32, kind="ExternalInput").ap()
    embd = nc.dram_tensor("emb", [G, E], F32, kind="ExternalInput").ap()
    wbd = nc.dram_tensor("wb", [FC_IN + 1], F32, kind="ExternalInput").ap()
    ys = nc.dram_tensor("ys", [PB, G], F32, kind="ExternalOutput").ap()

    add = mybir.AluOpType.add

    with tile.TileContext(nc) as tc, ExitStack() as ctx:
        const = ctx.enter_context(tc.tile_pool(name="const", bufs=1))
        psum = ctx.enter_context(tc.tile_pool(name="psum", bufs=8, space="PSUM"))

        # ---- W|b broadcast row, re-homed onto DVE ----
        wbc = const.tile([PB, FC_IN + 1], F32)
        nc.gpsimd.dma_start(
            out=wbc,
            in_=bass.AP(tensor=wbd.tensor, offset=0, ap=[[0, PB], [1, FC_IN + 1]]),
        )
        wscr = const.tile([PB, FC_IN + 1], F32)
        nc.vector.tensor_copy(wscr, wbc)
        wng = wscr[:, 0:DNG]                    # [128, 64]
        wx = wscr[:, DNG:DNG + 1]               # [128, 1]
        bias = wscr[:, FC_IN:FC_IN + 1]         # [128, 1]

        ind = const.tile([EP, EP], F32)
        gtp = const.tile([EP, EN], F32)

        # indicator ind[p, gg] = (p == gg); the matmul lhsT reads column
        # gg broadcast along the free dim via a stride-0 AP.
        iota_t = const.tile([EP, EP], mybir.dt.int32)
        nc.gpsimd.iota(
            iota_t,
            pattern=[[-1, EP]],
            base=0,
            channel_multiplier=1,
        )
        nc.gpsimd.tensor_scalar(
            out=ind,
            in0=iota_t,
            scalar1=0,
            scalar2=None,
            op0=mybir.AluOpType.is_equal,
        )

        # ngb[p] = sum_k x[p, G+k] * W_ng[k] + bias
        xng = const.tile([PB, DNG], F32)
        nc.sync.dma_start(out=xng, in_=xs[:, G:G + DNG])
        nc.vector.tensor_mul(xng, xng, wng)
        ng = const.tile([PB, 1], F32)
        nc.vector.tensor_reduce(ng, xng, axis=mybir.AxisListType.X, op=add)
        ngb = const.tile([PB, 1], F32)
        nc.vector.tensor_add(ngb, ng, bias)

        # gtp[gg, n] = sum_e emb[gg*EN + n, e] * W_e[e]
        # (loads on the ACT HWDGE ring, mult+reduce on DVE, two pipelined halves)
        eprep = ctx.enter_context(tc.tile_pool(name="eprep", bufs=2))
        emb_v = embd.rearrange("(p n) e -> p n e", p=EP)
        we_v = wscr[0:EP, DNG + 1:DNG + 1 + E].rearrange(
            "p (o e) -> p o e", o=1
        ).to_broadcast([EP, EN // 2, E])
        for h in range(2):
            n0 = h * (EN // 2)
            ehalf = eprep.tile([EP, EN // 2, E], F32, tag="ehalf")
            nc.scalar.dma_start(out=ehalf, in_=emb_v[:, n0:n0 + EN // 2, :])
            nc.vector.tensor_mul(ehalf, ehalf, we_v)
            nc.vector.tensor_reduce(
                gtp[:, n0:n0 + EN // 2], ehalf, axis=mybir.AxisListType.X, op=add
            )

        # ---- main stream: y = Identity(x * w_x + ngb) + broadcast(gene) ----
        xpool = ctx.enter_context(tc.tile_pool(name="xpool", bufs=5))
        ypool = ctx.enter_context(tc.tile_pool(name="ypool", bufs=G // DMA_COLS))
        for i in range(G // DMA_COLS):
            c0 = i * DMA_COLS
            x_t = xpool.tile([PB, DMA_COLS], F32, tag="x")
            nc.sync.dma_start(out=x_t, in_=xs[:, c0:c0 + DMA_COLS])
            y_t = ypool.tile([PB, DMA_COLS], F32, tag="y")
            for q in range(NQ):
                j0 = q * NT
                g = i * NQ + q
                cps = psum.tile([PB, NT], F32, tag="C")
                for k in range(2):
                    gg = 2 * g + k
                    nc.tensor.matmul(
                        cps[:, k * EN:(k + 1) * EN],
                        ind[:, gg:gg + 1].to_broadcast([EP, PB]),
                        gtp,
                        start=True,
                        stop=True,
                    )
                nc.scalar.activation(
                    out=y_t[:, j0:j0 + NT],
                    in_=x_t[:, j0:j0 + NT],
                    func=mybir.ActivationFunctionType.Identity,
                    bias=ngb,
                    scale=wx,
                )
                nc.vector.tensor_add(y_t[:, j0:j0 + NT], y_t[:, j0:j0 + NT], cps)
            nc.scalar.dma_start(out=ys[:, c0:c0 + DMA_COLS], in_=y_t)


def make_nc() -> bacc.Bacc:
    nc = bacc.Bacc("TRN2", debug=False, num_devices=NCORES)
    build_kernel(nc)
    nc.compile()  # legalizes sync waits (<=1 per instruction on TRN2)
    return nc


def kernel(**inputs) -> np.ndarray:
    x = np.ascontiguousarray(np.asarray(inputs["x"], dtype=np.float32))
    emb = np.ascontiguousarray(np.asarray(inputs["emb"], dtype=np.float32))
    W = np.asarray(inputs["W"], dtype=np.float32).reshape(FC_IN)
    b = np.asarray(inputs["b"], dtype=np.float32).reshape(1)
    wb = np.ascontiguousarray(np.concatenate([W, b]))

    nc = make_nc()
    in_maps = [
        {
            "xs": np.ascontiguousarray(x[c * PB:(c + 1) * PB]),
            "emb": emb,
            "wb": wb,
        }
        for c in range(NCORES)
    ]
    res = run_bass_kernel_spmd(nc, in_maps, core_ids=list(range(NCORES)))
    return np.concatenate([r["ys"] for r in res.results], axis=0)
